# Initial kernel scaffold
#
"""Causal self-attention (B=4, T=2048, D=1024, H=16) on 8 Trainium2 cores.

Sharding: core c handles batch b = c // 2 and head-half = c % 2 (8 of the 16
heads). Zero cross-core communication: each core computes q/k/v projections
for its 8 heads, causal flash-style attention, and a partial output
projection against its half of w_o. The host sums the two partial
projections per batch.

Layouts (every matmul operand is a direct slice, no on-device transposes):
  xT    (1024, 2048)  x[b].T            stationary slices for q/k/v
  wqT   (1024, 512)   w_q[rows].T       (softmax scale applied in the exp)
  wkT   (1024, 512)   w_k[rows].T
  wvT   (1024, 512)   w_v[rows].T
  woT   (512, 1024)   w_o[:, cols].T
  poutT (1024, 2048)  partial (x @ w_o.T contribution).T

Attention per head (dh=64): scores computed TRANSPOSED (k on partitions,
q on free dim) so softmax tiles feed P@V directly as the moving operand.
The two heads of a head-pair run as row-tiled concurrent matmuls
(tile_position (0,0)/(64,0)).  exp runs on the scalar engine with
scale=1/8 folded in; the denominator comes from a 65th all-ones column
appended to v; normalization broadcasts the sums across 64 partitions
with a K=1 ones matmul, then DVE reciprocal + multiply.

Causal structure: for q-block qb (512 wide) the k tiles 4qb..4qb+3 are
"diagonal": columns left of the triangle window are fully masked, so
scores / exp / P@V all skip them, and the GPSIMD affine_select mask only
touches the 128-wide triangle window.

Scheduling: one flat issue order.  The attention inner loops are
ACT(exp)-bound, so q/k projections for the NEXT head-pair and output
projection tiles are issued as small "filler" bursts inside the
attention slots to keep the tensor engine dense (HAM stays warm).

Everything 16-bit (fp16) into the PE; PSUM and the final partial output
stay fp32.
"""
import sys

if "/opt/trn_rl_repo" not in sys.path:
    sys.path.insert(0, "/opt/trn_rl_repo")

import numpy as np

B, T, D, H = 4, 2048, 1024, 16
P, TQ = 128, 512
ND = D // P          # 8  d-slices (contraction tiles for projections)
NHP = 4              # head-pairs per core (8 heads)
NQB = T // TQ        # 4  q blocks
NKB = T // P         # 16 k tiles

PRECISION = "fp16"

_COMPILED = {}


def _build(precision=None):
    import concourse.bacc as bacc
    import concourse.tile as tile
    from concourse import mybir
    from contextlib import ExitStack

    F32 = mybir.dt.float32
    F32R = mybir.dt.float32r
    F16 = mybir.dt.float16
    AF = mybir.ActivationFunctionType

    nc = bacc.Bacc("TRN2", target_bir_lowering=False, debug=False, num_devices=8)

    xT = nc.dram_tensor("xT", [D, T], F16, kind="ExternalInput")
    wqT = nc.dram_tensor("wqT", [D, 512], F16, kind="ExternalInput")
    wkT = nc.dram_tensor("wkT", [D, 512], F16, kind="ExternalInput")
    wvT = nc.dram_tensor("wvT", [D, 512], F16, kind="ExternalInput")
    woT = nc.dram_tensor("woT", [512, D], F16, kind="ExternalInput")
    pout = nc.dram_tensor("poutT", [D, T], F32, kind="ExternalOutput")

    with tile.TileContext(nc) as tc:
        with ExitStack() as ctx:
            xt_pool = ctx.enter_context(tc.tile_pool(name="xt", bufs=ND))
            w_pool = ctx.enter_context(tc.tile_pool(name="w", bufs=3 * ND + 1))
            wo_pool = ctx.enter_context(tc.tile_pool(name="wo", bufs=4))
            v_pool = ctx.enter_context(tc.tile_pool(name="v", bufs=NKB))
            q_pool = ctx.enter_context(tc.tile_pool(name="q", bufs=NHP))
            k_pool = ctx.enter_context(tc.tile_pool(name="k", bufs=NHP))
            ao_pool = ctx.enter_context(tc.tile_pool(name="ao", bufs=NHP))
            p_pool = ctx.enter_context(tc.tile_pool(name="p", bufs=4))
            r_pool = ctx.enter_context(tc.tile_pool(name="r", bufs=8))
            po_pool = ctx.enter_context(tc.tile_pool(name="po", bufs=4))
            mm_psum = ctx.enter_context(
                tc.tile_pool(name="mmps", bufs=2, space="PSUM"))
            s_psum = ctx.enter_context(
                tc.tile_pool(name="sps", bufs=2, space="PSUM"))
            o_psum = ctx.enter_context(
                tc.tile_pool(name="ops", bufs=2, space="PSUM"))

            xt = [xt_pool.tile([P, T], F16, tag="xt", name="xt")
                  for _ in range(ND)]
            wqs = [w_pool.tile([P, 512], F16, tag="w", name="wq")
                   for _ in range(ND)]
            wks = [w_pool.tile([P, 512], F16, tag="w", name="wk")
                   for _ in range(ND)]
            wvs = [w_pool.tile([P, 512], F16, tag="w", name="wv")
                   for _ in range(ND)]
            wos = [wo_pool.tile([P, D], F16, tag="wo", name="wo")
                   for _ in range(4)]
            vA = [v_pool.tile([P, 8, 65], F16, tag="vA", name="vA")
                  for _ in range(NKB)]
            qT = [q_pool.tile([P, T], F16, tag="qT", name="qT")
                  for _ in range(NHP)]
            kT = [k_pool.tile([P, T], F16, tag="kT", name="kT")
                  for _ in range(NHP)]
            aoT = [ao_pool.tile([P, T], F16, tag="aoT", name="aoT")
                   for _ in range(NHP)]
            ones_col = w_pool.tile([P, 8, 1], F16, tag="ones", name="ones")

            # ---------------- input DMA ----------------
            # x arrives in 512-column chunks so v-projection can start after
            # ~1/4 of x is resident; v weights ride along with the first
            # chunk, q/k/o weights after the rest of x.
            for ds in range(ND):
                nc.sync.dma_start(wvs[ds], wvT[ds * P:(ds + 1) * P, :])
                nc.sync.dma_start(xt[ds][:, 0:TQ],
                                  xT[ds * P:(ds + 1) * P, 0:TQ])
            for cc in range(1, 4):
                for ds in range(ND):
                    nc.sync.dma_start(
                        xt[ds][:, cc * TQ:(cc + 1) * TQ],
                        xT[ds * P:(ds + 1) * P, cc * TQ:(cc + 1) * TQ])
                if cc < 3:
                    for w_dram, wts in ((wqT, wqs), (wkT, wks)):
                        for ds in range(4 * (cc - 1), 4 * cc):
                            nc.sync.dma_start(
                                wts[ds], w_dram[ds * P:(ds + 1) * P, :])
            for cs in range(4):
                nc.sync.dma_start(wos[cs], woT[cs * P:(cs + 1) * P, :])
            nc.vector.memset(ones_col[:], 1.0)
            ones1 = r_pool.tile([1, 64], F32R, tag="ones1", name="ones1")
            ones1f = r_pool.tile([1, 64], F32, tag="ones1f", name="ones1f")
            nc.vector.memset(ones1f[:], 1.0)
            nc.vector.tensor_copy(ones1[:], ones1f[:])

            # ---------------- projection unit machinery ----------------
            # v-chunk: two k-position tiles, each accumulating 8
            # contraction steps in a PSUM tile (16 matmuls, atomic).
            def v_chunk(kc):
                def go():
                    ps = [mm_psum.tile([P, TQ], F32, tag="mm", name="vmm")
                          for _ in range(2)]
                    for ds in range(ND):
                        for i in range(2):
                            kb = 2 * kc + i
                            nc.tensor.matmul(
                                ps[i],
                                xt[ds][:, kb * P:(kb + 1) * P],
                                wvs[ds][:],
                                start=(ds == 0), stop=(ds == ND - 1))
                    for i in range(2):
                        kb = 2 * kc + i
                        nc.vector.tensor_copy(
                            vA[kb][:, :, 0:64],
                            ps[i][:].rearrange("p (h c) -> p h c", c=64))
                        nc.vector.tensor_copy(
                            vA[kb][:, :, 64:65], ones_col[:])
                return go

            # q/k unit: one (w, hp, tt) output tile, split in two 4-matmul
            # halves so it can be spread across two fill points.  At most
            # one other mm-pool allocation may occur between the halves
            # (the 2-buffer rotation then stays clear of the held tile);
            # the norm below was shaped to respect that.
            def qk_halves(wts, dst, hp, tt):
                cell = {}

                def first():
                    cell["ps"] = mm_psum.tile([P, TQ], F32, tag="mm",
                                              name="qkmm")
                    for ds in range(4):
                        nc.tensor.matmul(
                            cell["ps"],
                            wts[ds][:, hp * P:(hp + 1) * P],
                            xt[ds][:, tt * TQ:(tt + 1) * TQ],
                            start=(ds == 0), stop=False)

                def second():
                    ps = cell.pop("ps")
                    for ds in range(4, ND):
                        nc.tensor.matmul(
                            ps,
                            wts[ds][:, hp * P:(hp + 1) * P],
                            xt[ds][:, tt * TQ:(tt + 1) * TQ],
                            start=False, stop=(ds == ND - 1))
                    nc.vector.tensor_copy(
                        dst[:, tt * TQ:(tt + 1) * TQ], ps[:])

                return [first, second]

            def outproj_group(od, tt):
                def go():
                    ps = mm_psum.tile([P, TQ], F32, tag="mm", name="pomm")
                    for cs in range(4):
                        nc.tensor.matmul(
                            ps,
                            wos[cs][:, od * P:(od + 1) * P],
                            aoT[cs][:, tt * TQ:(tt + 1) * TQ],
                            start=(cs == 0), stop=(cs == 3))
                    po = po_pool.tile([P, TQ], F32, tag="po", name="po")
                    nc.vector.tensor_copy(po[:], ps[:])
                    nc.sync.dma_start(
                        pout[od * P:(od + 1) * P, tt * TQ:(tt + 1) * TQ],
                        po[:])
                return go

            # filler: deadline-tagged PE bursts issued inside attention
            # slots.  deadline = first global slot index (hp*4+qb) whose
            # attention requires the unit's output.  When the projection
            # supply runs out, fall through to out-projection groups.
            filler = []
            for hp in range(NHP):
                for tt in range(NQB):
                    if hp == 0 and tt < 2:
                        continue  # issued upfront below
                    for h in qk_halves(wqs, qT[hp], hp, tt):
                        filler.append((4 * hp + tt, h))
                    for h in qk_halves(wks, kT[hp], hp, tt):
                        filler.append((4 * hp + tt, h))
            filler.sort(key=lambda e: e[0])
            fill_pos = [0]
            out_groups = []
            out_pos = [0]

            def consume_filler(n):
                for _ in range(n):
                    if fill_pos[0] < len(filler):
                        filler[fill_pos[0]][1]()
                        fill_pos[0] += 1
                    elif out_pos[0] < len(out_groups):
                        out_groups[out_pos[0]]()
                        out_pos[0] += 1
                    else:
                        break

            def consume_due(slot_idx):
                while fill_pos[0] < len(filler) and \
                        filler[fill_pos[0]][0] <= slot_idx:
                    filler[fill_pos[0]][1]()
                    fill_pos[0] += 1

            def consume_outproj(n):
                a = out_pos[0]
                b = min(a + n, len(out_groups))
                for i in range(a, b):
                    out_groups[i]()
                out_pos[0] = b

            # upfront: all of v, and q/k for (hp=0, tt=0..1); later q/k
            # tiles stream in as attention-slot filler.
            for kc in range(NKB // 2):
                v_chunk(kc)()
            for tt in range(2):
                for unit in (qk_halves(wqs, qT[0], 0, tt),
                             qk_halves(wks, kT[0], 0, tt)):
                    for h in unit:
                        h()

            # ---------------- attention ----------------
            # each slot's normalization is deferred into the next slot so
            # the ~3us DVE->PE->DVE chain runs while the PE streams the next
            # slot's scores; o_psum rotation (bufs=2) then unblocks in time.
            pending_norm = [None]

            def attention_slot(hp, qb):
                consume_due(4 * hp + qb)
                diag = [4 * qb + i for i in range(4)]
                order = diag + list(range(4 * qb))
                n_kb = len(order)
                o_ps = [o_psum.tile([P, TQ], F32, tag="o", name="o")
                        for _ in range(2)]
                s_tiles = {}
                pts = {}

                def issue_scores(kb):
                    c0 = max(0, kb * P - qb * TQ)
                    sp = s_psum.tile([P, 2, TQ], F32, tag="s", name="s")
                    for j in range(2):
                        nc.tensor.matmul(
                            sp[:, j, c0:TQ],
                            kT[hp][j * 64:(j + 1) * 64, kb * P:(kb + 1) * P],
                            qT[hp][j * 64:(j + 1) * 64,
                                   qb * TQ + c0:(qb + 1) * TQ],
                            tile_position=(j * 64, 0))
                    s_tiles[kb] = (sp, c0)

                def issue_exp(kb):
                    sp, c0 = s_tiles.pop(kb)
                    pt = p_pool.tile([P, 2, TQ], F16, tag="p", name="p")
                    nc.scalar.activation(pt[:, :, c0:TQ], sp[:, :, c0:TQ],
                                         AF.Exp, scale=0.125)
                    if kb >= 4 * qb:
                        # triangle window of the diagonal tile
                        nc.gpsimd.affine_select(
                            out=pt[:, :, c0:c0 + P], in_=pt[:, :, c0:c0 + P],
                            pattern=[[0, 2], [1, P]],
                            compare_op=mybir.AluOpType.is_ge,
                            fill=0.0, base=0, channel_multiplier=-1)
                    pts[kb] = (pt, c0)

                def issue_pv(kb, first, last):
                    pt, c0 = pts.pop(kb)
                    for j in range(2):
                        nc.tensor.matmul(
                            o_ps[j][0:65, c0:TQ],
                            vA[kb][:, 2 * hp + j, :],
                            pt[:, j, c0:TQ],
                            start=first, stop=last)

                issue_scores(order[0])
                issue_scores(order[1])
                if pending_norm[0] is not None:
                    consume_filler(1)
                    pending_norm[0]()
                    pending_norm[0] = None
                issue_exp(order[0])
                consume_filler(1)
                for i, kb in enumerate(order):
                    if i + 2 < n_kb:
                        issue_scores(order[i + 2])
                    if i + 1 < n_kb:
                        issue_exp(order[i + 1])
                    issue_pv(kb, first=(i == 0), last=(i == n_kb - 1))
                    if i % 4 == 3:
                        consume_filler(1)

                # normalization: rows 0..63 of o_ps = unnormalized out.T,
                # row 64 = sum(exp).  Sums copied out, then their
                # reciprocal is broadcast across 64 partitions via a K=1
                # ones matmul into the unused upper half of the o_ps bank,
                # and the multiply writes aoT.
                def norm():
                    for j in range(2):
                        sc = r_pool.tile([1, TQ], F32R, tag="sc", name="sc")
                        nc.vector.tensor_copy(sc[:], o_ps[j][64:65, :])
                        rb = mm_psum.tile([P, TQ], F32, tag="mm", name="rb")
                        nc.tensor.matmul(rb[0:64, :], ones1[:], sc[:],
                                         start=True, stop=True)
                        R = r_pool.tile([64, TQ], F32, tag="Rb", name="R")
                        nc.vector.reciprocal_approx_fast(R[:], rb[0:64, :])
                        nc.vector.tensor_mul(
                            aoT[hp][j * 64:(j + 1) * 64,
                                    qb * TQ:(qb + 1) * TQ],
                            o_ps[j][0:64, :], R[:])
                pending_norm[0] = norm

            # out-projection groups for tt become available once hp=3's
            # norm for qb=tt has been issued (inside the NEXT slot, since
            # norms are deferred).
            for hp in range(NHP):
                for qb in range(NQB):
                    attention_slot(hp, qb)
                    if hp == 3 and qb >= 1:
                        out_groups.extend(
                            outproj_group(od, qb - 1) for od in range(ND))
                        consume_outproj(8)
            pending_norm[0]()
            pending_norm[0] = None
            out_groups.extend(outproj_group(od, 3) for od in range(ND))
            consume_filler(len(filler) - fill_pos[0])
            consume_outproj(len(out_groups))

    nc.compile()
    return nc


def _get_compiled(precision=None):
    key = precision or PRECISION
    if key not in _COMPILED:
        _COMPILED[key] = _build(key)
    return _COMPILED[key]


def make_in_maps(x, w_q, w_k, w_v, w_o, precision=None):
    xTs = [np.ascontiguousarray(x[b].T).astype(np.float16) for b in range(B)]
    wq = [np.ascontiguousarray(w_q[h * 512:(h + 1) * 512].T).astype(np.float16)
          for h in range(2)]
    wk = [np.ascontiguousarray(w_k[h * 512:(h + 1) * 512].T).astype(np.float16)
          for h in range(2)]
    wv = [np.ascontiguousarray(w_v[h * 512:(h + 1) * 512].T).astype(np.float16)
          for h in range(2)]
    wo = [np.ascontiguousarray(w_o[:, h * 512:(h + 1) * 512].T).astype(np.float16)
          for h in range(2)]
    in_maps = []
    for c in range(8):
        b, half = divmod(c, 2)
        in_maps.append({
            "xT": xTs[b],
            "wqT": wq[half],
            "wkT": wk[half],
            "wvT": wv[half],
            "woT": wo[half],
        })
    return in_maps


def kernel(x, w_q, w_k, w_v, w_o):
    from concourse.bass_utils import run_bass_kernel_spmd

    x = np.asarray(x, dtype=np.float32)
    w_q = np.asarray(w_q, dtype=np.float32)
    w_k = np.asarray(w_k, dtype=np.float32)
    w_v = np.asarray(w_v, dtype=np.float32)
    w_o = np.asarray(w_o, dtype=np.float32)

    nc = _get_compiled()
    in_maps = make_in_maps(x, w_q, w_k, w_v, w_o)
    res = run_bass_kernel_spmd(nc, in_maps, list(range(8)))

    out = np.empty((B, T, D), dtype=np.float32)
    for b in range(B):
        out[b] = (res.results[2 * b]["poutT"] + res.results[2 * b + 1]["poutT"]).T
    return out



# revision 64
# speedup vs baseline: 1.0802x; 1.0802x over previous
"""Causal self-attention (B=4, T=2048, D=1024, H=16) on 8 Trainium2 cores.

Sharding: core c handles batch b = c // 2 and head-half = c % 2 (8 of the 16
heads). Zero cross-core communication: each core computes q/k/v projections
for its 8 heads, causal flash-style attention, and a partial output
projection against its half of w_o. The host sums the two partial
projections per batch.

Layouts (partition-major so every input is 1-2 large DMAs):
  xT    (128, 8, 2048)  x[b].T as (p, ds, t)       stationary slices
  wqT   (128, 8, 512)   w_q[rows].T as (p, ds, f)
  wkT   (128, 8, 512)
  wvT   (128, 8, 512)
  woT   (128, 4, 1024)  w_o[:, cols].T as (p, cs, od)
  poutT (128, 8, 2048)  partial (x @ w_o.T).T as (p, od, t), fp16

Attention per head (dh=64): scores computed TRANSPOSED (k on partitions,
q on free dim) so softmax tiles feed P@V directly as the moving operand.
The two heads of a head-pair run as row-tiled matmuls (tile_position
(0,0)/(64,0)).  exp runs on the scalar engine with scale=1/8 folded in;
the denominator comes from a 65th all-ones column appended to v; the
normalization takes a 1-row DVE reciprocal of the sums, broadcasts it
across 64 partitions on GPSIMD (partition_broadcast), and multiplies on
DVE.  No tensor-engine work in the normalization.

Causal structure: for q-block qb (512 wide) the k tiles 4qb..4qb+3 are
"diagonal": columns left of the triangle window are fully masked, so
scores / exp / P@V all skip them, and the GPSIMD affine_select mask only
touches the 128-wide triangle window.

Scheduling: one flat issue order.  The attention inner loops are
ACT(exp)-bound, so q/k projections for the NEXT head-pair and output
projection tiles are issued as small "filler" bursts inside the
attention slots to keep the tensor engine dense (HAM stays warm).

Everything 16-bit (fp16) into the PE; PSUM stays fp32, the partial
output is stored fp16 and summed on the host in fp32.
"""
import sys

if "/opt/trn_rl_repo" not in sys.path:
    sys.path.insert(0, "/opt/trn_rl_repo")

import numpy as np

B, T, D, H = 4, 2048, 1024, 16
P, TQ = 128, 512
ND = D // P          # 8  d-slices (contraction tiles for projections)
NHP = 4              # head-pairs per core (8 heads)
NQB = T // TQ        # 4  q blocks
NKB = T // P         # 16 k tiles

PRECISION = "fp16"

_COMPILED = {}


def _build(precision=None, debug_ao=False):
    import concourse.bacc as bacc
    import concourse.tile as tile
    from concourse import mybir
    from contextlib import ExitStack

    F32 = mybir.dt.float32
    F32R = mybir.dt.float32r
    F16 = mybir.dt.float16
    AF = mybir.ActivationFunctionType

    nc = bacc.Bacc("TRN2", target_bir_lowering=False, debug=False, num_devices=8)

    xT = nc.dram_tensor("xT", [P, ND, T], F16, kind="ExternalInput")
    wqT = nc.dram_tensor("wqT", [P, ND, 512], F16, kind="ExternalInput")
    wkT = nc.dram_tensor("wkT", [P, ND, 512], F16, kind="ExternalInput")
    wvT = nc.dram_tensor("wvT", [P, ND, 512], F16, kind="ExternalInput")
    woT = nc.dram_tensor("woT", [P, 4, D], F16, kind="ExternalInput")
    pout = nc.dram_tensor("poutT", [P, ND, T], F16, kind="ExternalOutput")
    ao_dump = (nc.dram_tensor("aoDump", [P, 3, T], F16, kind="ExternalOutput")
               if debug_ao else None)
    va_dump = (nc.dram_tensor("vaDump", [P, NKB, 8, P], F16, kind="ExternalOutput")
               if debug_ao else None)


    with tile.TileContext(nc) as tc:
        with ExitStack() as ctx:
            big_pool = ctx.enter_context(tc.tile_pool(name="big", bufs=1))
            p_pool = ctx.enter_context(tc.tile_pool(name="p", bufs=4))
            r_pool = ctx.enter_context(tc.tile_pool(name="r", bufs=4))
            po_pool = ctx.enter_context(tc.tile_pool(name="po", bufs=2))
            mm_psum = ctx.enter_context(
                tc.tile_pool(name="mmps", bufs=2, space="PSUM"))
            s_psum = ctx.enter_context(
                tc.tile_pool(name="sps", bufs=2, space="PSUM"))
            o_psum = ctx.enter_context(
                tc.tile_pool(name="ops", bufs=2, space="PSUM"))

            xt = big_pool.tile([P, ND, T], F16, tag="xt", name="xt")
            wq = big_pool.tile([P, ND, 512], F16, tag="wq", name="wq")
            wk = big_pool.tile([P, ND, 512], F16, tag="wk", name="wk")
            wv = big_pool.tile([P, ND, 512], F16, tag="wv", name="wv")
            wo = big_pool.tile([P, 4, D], F16, tag="wo", name="wo")
            # vA columns 64:128 are all-ones: P@V then leaves the exp-sums
            # replicated on PSUM rows 64:128, so normalization needs no
            # partition broadcast at all (columns of the moving operand set
            # the matmul cost, so the extra ones-columns are free).
            vA = big_pool.tile([P, NKB, 8, P], F16, tag="vA", name="vA")
            qT = big_pool.tile([P, NHP, T], F16, tag="qT", name="qT")
            kT = big_pool.tile([P, NHP, T], F16, tag="kT", name="kT")
            # hp=3 gets its own tile so the final out-projection's
            # stage-A reads (heads 0..2) carry no dependency edge to the
            # last slot's norm write
            aoT = big_pool.tile([P, 3, T], F16, tag="aoT", name="aoT")
            aoT3 = big_pool.tile([P, T], F16, tag="aoT3", name="aoT3")

            # ---------------- input DMA ----------------
            # Large partition-major transfers; wv + first x chunk split
            # so the very first v matmul can start after ~256KB.
            for ds in range(4):
                nc.sync.dma_start(wv[:, ds:ds + 1, :], wvT[:, ds:ds + 1, :])
                nc.sync.dma_start(xt[:, ds:ds + 1, 0:TQ],
                                  xT[:, ds:ds + 1, 0:TQ])
            nc.sync.dma_start(wv[:, 4:8, :], wvT[:, 4:8, :])
            nc.sync.dma_start(xt[:, 4:8, 0:TQ], xT[:, 4:8, 0:TQ])
            nc.sync.dma_start(xt[:, :, TQ:2 * TQ], xT[:, :, TQ:2 * TQ])
            nc.sync.dma_start(wq[:], wqT[:])
            nc.sync.dma_start(xt[:, :, 2 * TQ:3 * TQ], xT[:, :, 2 * TQ:3 * TQ])
            nc.sync.dma_start(wk[:], wkT[:])
            nc.sync.dma_start(xt[:, :, 3 * TQ:4 * TQ], xT[:, :, 3 * TQ:4 * TQ])
            nc.sync.dma_start(wo[:], woT[:])
            # ones in columns 0:64 (so exp-sums land at PSUM rows 0:64,
            # where the custom-DVE reciprocal works), values at 64:128
            nc.vector.memset(vA[:, :, :, 0:64], 1.0)

            # ---------------- projection unit machinery ----------------
            # v super-chunk: four k-position tiles accumulating 8
            # contraction steps each, spread over both the mm and the
            # (pre-attention idle) o PSUM pools.  ds-outer order means the
            # PE consumes input DMAs slice by slice instead of stalling
            # for a full chunk, and 4 tiles in flight hide the copies.
            def v_chunk(sc):
                def go():
                    ps = [mm_psum.tile([P, TQ], F32, tag="mm", name="vmm"),
                          mm_psum.tile([P, TQ], F32, tag="mm", name="vmm"),
                          o_psum.tile([P, TQ], F32, tag="o", name="vmm"),
                          o_psum.tile([P, TQ], F32, tag="o", name="vmm")]
                    for ds in range(ND):
                        for i in range(4):
                            kb = 4 * sc + i
                            nc.tensor.matmul(
                                ps[i],
                                xt[:, ds, kb * P:(kb + 1) * P],
                                wv[:, ds, :],
                                start=(ds == 0), stop=(ds == ND - 1))
                    for i in range(4):
                        kb = 4 * sc + i
                        nc.vector.tensor_copy(
                            vA[:, kb, :, 64:P],
                            ps[i][:].rearrange("p (h c) -> p h c", c=64))
                return go

            # q/k unit: one (w, hp, tt) output tile, split in two 4-matmul
            # halves so it can be spread across two fill points.  At most
            # one other mm-pool allocation may occur between the halves
            # (the 2-buffer rotation then stays clear of the held tile).
            def qk_halves(wts, dst, hp, tt):
                cell = {}

                def first():
                    cell["ps"] = mm_psum.tile([P, TQ], F32, tag="mm",
                                              name="qkmm")
                    for ds in range(4):
                        nc.tensor.matmul(
                            cell["ps"],
                            wts[:, ds, hp * P:(hp + 1) * P],
                            xt[:, ds, tt * TQ:(tt + 1) * TQ],
                            start=(ds == 0), stop=False)

                def second():
                    ps = cell.pop("ps")
                    for ds in range(4, ND):
                        nc.tensor.matmul(
                            ps,
                            wts[:, ds, hp * P:(hp + 1) * P],
                            xt[:, ds, tt * TQ:(tt + 1) * TQ],
                            start=False, stop=(ds == ND - 1))
                    nc.vector.tensor_copy(
                        dst[:, hp, tt * TQ:(tt + 1) * TQ], ps[:])

                return [first, second]

            po_tiles = {}

            def outproj_group(od, tt):
                def go():
                    ps = mm_psum.tile([P, TQ], F32, tag="mm", name="pomm")
                    for cs in range(4):
                        mov = (aoT3[:, tt * TQ:(tt + 1) * TQ] if cs == 3
                               else aoT[:, cs, tt * TQ:(tt + 1) * TQ])
                        nc.tensor.matmul(
                            ps,
                            wo[:, cs, od * P:(od + 1) * P],
                            mov,
                            start=(cs == 0), stop=(cs == 3))
                    if tt not in po_tiles:
                        po_tiles[tt] = po_pool.tile(
                            [P, ND, TQ], F16, tag="po", name="po")
                    po = po_tiles[tt]
                    nc.vector.tensor_copy(po[:, od, :], ps[:])
                    if tt == NQB - 1:
                        # final block: per-od DMAs so the tail transfer
                        # after the last matmul stays small
                        nc.sync.dma_start(
                            pout[:, od, tt * TQ:(tt + 1) * TQ], po[:, od, :])
                    elif od == ND - 1:
                        nc.sync.dma_start(
                            pout[:, :, tt * TQ:(tt + 1) * TQ], po[:])
                        del po_tiles[tt]
                return go

            # filler: deadline-tagged PE bursts issued inside attention
            # slots.  deadline = first global slot index (hp*4+qb) whose
            # attention requires the unit's output.  When the projection
            # supply runs out, fall through to out-projection groups.
            # slots run qb-outer / hp-inner (slot_idx = 4*qb + hp) so the
            # out-projection for tt=qb unlocks right after each qb round —
            # filler supply then spans the whole attention phase.
            filler = []
            for hp in range(NHP):
                for tt in range(NQB):
                    if tt == 0 and hp < 2:
                        continue  # issued upfront below
                    # deadline 2 slots early so pacing pulls units in
                    # before consume_due would force a pipeline-draining
                    # burst at the slot that needs them
                    dl = max(0, 4 * tt + hp - 2)
                    for w_t, d_t in ((wq, qT), (wk, kT)):
                        first, second = qk_halves(w_t, d_t, hp, tt)
                        filler.append((dl, first, False))
                        filler.append((dl, second, True))
            filler.sort(key=lambda e: e[0])
            fill_pos = [0]
            out_groups = []
            out_pos = [0]

            def consume_filler(n):
                for _ in range(n):
                    if fill_pos[0] < len(filler):
                        filler[fill_pos[0]][1]()
                        fill_pos[0] += 1
                    elif out_pos[0] < len(out_groups):
                        out_groups[out_pos[0]]()
                        out_pos[0] += 1
                    else:
                        break

            def consume_due(slot_idx):
                while fill_pos[0] < len(filler) and \
                        filler[fill_pos[0]][0] <= slot_idx:
                    filler[fill_pos[0]][1]()
                    fill_pos[0] += 1

            def consume_outproj(n):
                # an out-projection's mm allocation must never land
                # between the two halves of a held q/k unit (2-buffer
                # rotation would alias the held accumulator): flush a
                # pending second half first.
                if fill_pos[0] < len(filler) and filler[fill_pos[0]][2]:
                    filler[fill_pos[0]][1]()
                    fill_pos[0] += 1
                a = out_pos[0]
                b = min(a + n, len(out_groups))
                for i in range(a, b):
                    out_groups[i]()
                out_pos[0] = b

            # upfront: all of v, and q/k for (hp=0..1, tt=0); later q/k
            # tiles stream in as attention-slot filler.
            for sc in range(NKB // 4):
                v_chunk(sc)()
            for hp in range(2):
                for unit in (qk_halves(wq, qT, hp, 0),
                             qk_halves(wk, kT, hp, 0)):
                    for h in unit:
                        h()

            # ---------------- attention ----------------
            def attention_slot(hp, qb):
                consume_due(4 * qb + hp)
                # first tile must be full-width (c0=0, start=True); the
                # other diagonal tiles go LAST, smallest exp at the end,
                # so the next slot's scores wait on a short exp for their
                # s-buffer instead of a full 1.1us one.
                order = ([4 * qb] + list(range(4 * qb))
                         + [4 * qb + 1, 4 * qb + 2, 4 * qb + 3])
                n_kb = len(order)
                o_ps = [o_psum.tile([P, TQ], F32, tag="o", name="o")
                        for _ in range(2)]
                s_tiles = {}
                pts = {}

                def issue_scores(kb):
                    c0 = max(0, kb * P - qb * TQ)
                    sp = s_psum.tile([P, 2, TQ], F32, tag="s", name="s")
                    for j in range(2):
                        nc.tensor.matmul(
                            sp[:, j, c0:TQ],
                            kT[j * 64:(j + 1) * 64, hp, kb * P:(kb + 1) * P],
                            qT[j * 64:(j + 1) * 64, hp,
                               qb * TQ + c0:(qb + 1) * TQ],
                            tile_position=(j * 64, 0))
                    s_tiles[kb] = (sp, c0)

                def issue_exp(kb):
                    sp, c0 = s_tiles.pop(kb)
                    pt = p_pool.tile([P, 2, TQ], F16, tag="p", name="p")
                    nc.scalar.activation(pt[:, :, c0:TQ], sp[:, :, c0:TQ],
                                         AF.Exp, scale=0.125)
                    if kb >= 4 * qb:
                        # triangle window of the diagonal tile
                        nc.gpsimd.affine_select(
                            out=pt[:, :, c0:c0 + P], in_=pt[:, :, c0:c0 + P],
                            pattern=[[0, 2], [1, P]],
                            compare_op=mybir.AluOpType.is_ge,
                            fill=0.0, base=0, channel_multiplier=-1)
                    pts[kb] = (pt, c0)

                def issue_pv(kb, first, last):
                    pt, c0 = pts.pop(kb)
                    for j in range(2):
                        nc.tensor.matmul(
                            o_ps[j][:, c0:TQ],
                            vA[:, kb, 2 * hp + j, :],
                            pt[:, j, c0:TQ],
                            start=first, stop=last)

                # PE work in a slot falls short of its exp time by
                # ~(2us*qb + 1.7us); feed filler at that rate (one unit
                # ~= 0.86us) so the in-order PE queue never drains — every
                # idle gap also costs a p-state ramp re-climb.
                consume_filler(1)
                issue_scores(order[0])
                issue_scores(order[1])
                issue_exp(order[0])
                target = (2, 3, 6, 8)[qb]
                for i, kb in enumerate(order):
                    # filler goes BEFORE the next scores matmul: the PE
                    # queue is in-order and scores(i+2) blocks on exp(i)
                    # freeing its s_psum buffer, so ready projection work
                    # must be queued ahead of that wait to cover it.
                    if (i + 1) * target // n_kb > i * target // n_kb:
                        consume_filler(1)
                    if i + 2 < n_kb:
                        issue_scores(order[i + 2])
                    if i + 1 < n_kb:
                        issue_exp(order[i + 1])
                    issue_pv(kb, first=(i == 0), last=(i == n_kb - 1))

                # normalization (issued right away — it is pure DVE now):
                # rows 0..63 of o_ps = sum(exp) replicated (ones-columns
                # of vA sit first), rows 64..127 = unnormalized out.T.
                # Reciprocal straight off PSUM rows 0:64, then multiply.
                for j in range(2):
                    R = r_pool.tile([64, TQ], F32, tag="Rb", name="R")
                    nc.vector.reciprocal_approx_fast(R[:], o_ps[j][0:64, :])
                    dst = (aoT3[j * 64:(j + 1) * 64, qb * TQ:(qb + 1) * TQ]
                           if hp == 3 else
                           aoT[j * 64:(j + 1) * 64, hp,
                               qb * TQ:(qb + 1) * TQ])
                    nc.vector.tensor_mul(dst, o_ps[j][64:P, :], R[:])

            # out-projection groups for tt=qb become available once the
            # last head-pair's norm for qb has been issued.
            for qb in range(NQB):
                for hp in range(NHP):
                    attention_slot(hp, qb)
                if qb < NQB - 1:
                    out_groups.extend(
                        outproj_group(od, qb) for od in range(ND))
            consume_filler(len(filler) - fill_pos[0])
            consume_outproj(len(out_groups))

            # final block tt=3: two-stage software pipeline.  Stage A
            # (cs=0..2) is independent of the just-issued last norm; the
            # accumulators cycle over mm plus the now-idle s_psum banks so
            # four stage-A bursts can run while the norm chain drains.
            ps3 = {}

            def out3_a(od):
                # od 0,1 from the mm pool: the o pool is still held by the
                # last slot's norm — exactly the latency stage A covers.
                if (od // 2) % 2 == 0:
                    ps = mm_psum.tile([P, TQ], F32, tag="mm", name="o3")
                else:
                    ps = o_psum.tile([P, TQ], F32, tag="o", name="o3")
                ps3[od] = ps
                for cs in range(3):
                    nc.tensor.matmul(
                        ps,
                        wo[:, cs, od * P:(od + 1) * P],
                        aoT[:, cs, 3 * TQ:4 * TQ],
                        start=(cs == 0), stop=False)

            def out3_b(od):
                ps = ps3.pop(od)
                nc.tensor.matmul(
                    ps,
                    wo[:, 3, od * P:(od + 1) * P],
                    aoT3[:, 3 * TQ:4 * TQ],
                    start=False, stop=True)
                if 3 not in po_tiles:
                    po_tiles[3] = po_pool.tile(
                        [P, ND, TQ], F16, tag="po", name="po")
                po = po_tiles[3]
                nc.vector.tensor_copy(po[:, od, :], ps[:])
                nc.sync.dma_start(pout[:, od, 3 * TQ:4 * TQ], po[:, od, :])

            out3_a(0)
            out3_a(1)
            out3_a(2)
            out3_a(3)
            for od in range(ND):
                out3_b(od)
                if od + 4 < ND:
                    out3_a(od + 4)
            if ao_dump is not None:
                nc.sync.dma_start(ao_dump[:], aoT[:])
                nc.sync.dma_start(va_dump[:], vA[:])

    nc.compile()
    return nc


def _get_compiled(precision=None):
    key = precision or PRECISION
    if key not in _COMPILED:
        _COMPILED[key] = _build(key)
    return _COMPILED[key]


def _pm(a, nd):
    # (nd*128, cols) -> partition-major (128, nd, cols) fp16
    return np.ascontiguousarray(
        a.reshape(nd, P, -1).transpose(1, 0, 2)).astype(np.float16)


def make_in_maps(x, w_q, w_k, w_v, w_o, precision=None):
    xTs = [_pm(np.asarray(x[b]).T, ND) for b in range(B)]
    wq = [_pm(w_q[h * 512:(h + 1) * 512].T, ND) for h in range(2)]
    wk = [_pm(w_k[h * 512:(h + 1) * 512].T, ND) for h in range(2)]
    wv = [_pm(w_v[h * 512:(h + 1) * 512].T, ND) for h in range(2)]
    wo = [_pm(w_o[:, h * 512:(h + 1) * 512].T, 4) for h in range(2)]
    in_maps = []
    for c in range(8):
        b, half = divmod(c, 2)
        in_maps.append({
            "xT": xTs[b],
            "wqT": wq[half],
            "wkT": wk[half],
            "wvT": wv[half],
            "woT": wo[half],
        })
    return in_maps


def kernel(x, w_q, w_k, w_v, w_o):
    from concourse.bass_utils import run_bass_kernel_spmd

    x = np.asarray(x, dtype=np.float32)
    w_q = np.asarray(w_q, dtype=np.float32)
    w_k = np.asarray(w_k, dtype=np.float32)
    w_v = np.asarray(w_v, dtype=np.float32)
    w_o = np.asarray(w_o, dtype=np.float32)

    nc = _get_compiled()
    in_maps = make_in_maps(x, w_q, w_k, w_v, w_o)
    res = run_bass_kernel_spmd(nc, in_maps, list(range(8)))

    out = np.empty((B, T, D), dtype=np.float32)
    for b in range(B):
        pm = (res.results[2 * b]["poutT"].astype(np.float32)
              + res.results[2 * b + 1]["poutT"].astype(np.float32))
        out[b] = pm.transpose(1, 0, 2).reshape(D, T).T
    return out
